# revision 9
# baseline (speedup 1.0000x reference)
"""Trainium2 Bass kernel for nn_Cond_Attn (B=4, C=256, N=4096, d=8).

Sharding: 8 cores; core i handles batch i//2, attention-row half i%2
(2048 of 4096 rows). Each core computes its full attention row-slice
(softmax over all 4096 keys is local) plus the matching out columns.

Per-core pipeline (all fp32):
  - pq (8,2048), pk (8,4096): tiny projections via PE (weights passed
    pre-transposed from host; biases folded in as K=1 matmuls).
  - pvT_aug (4096, 257): value projection, transposed layout, with a
    ones column at index 256 so the softmax denominator S falls out of
    the OT matmul for free.
  - per 512-wide n-chunk: energyT tiles (m,n) -> ACT exp -> OT matmuls
    (lhsT = expET 128x128 subtiles, rhs = pvT_aug) accumulate
    OT (n,257) in PSUM: cols 0:256 = unnormalized out^T, col 256 = S.
  - per 128-row n-tile: recompute energy in (n,m) layout and apply
    exp(E - lnS) on ACT (per-partition bias) -> normalized attention,
    DMA'd out in 1MB blocks. OT is scaled by gamma/S, PE-transposed to
    (c,n), residual x added, DMA'd out.
"""
import sys

if "/opt/trn_rl_repo" not in sys.path:
    sys.path.insert(0, "/opt/trn_rl_repo")

import json
from contextlib import ExitStack

import numpy as np

B, C, N, D = 4, 256, 4096, 8
NSL = N // 2          # n rows per core
NCH = 512             # n-chunk width
NT = 128              # n-tile (partition block)
MT = 128              # m-tile
E1 = 257              # pvT width incl. ones column

_CACHE = {}


def _build(gamma: float, loop_iters: int | None = None):
    import concourse.bass as bass
    import concourse.mybir as mybir
    import concourse.tile as tile
    from concourse.masks import make_identity

    fp32 = mybir.dt.float32
    AF = mybir.ActivationFunctionType

    nc = bass.Bass()
    if loop_iters is None:
        qs = nc.declare_dram_parameter("qs", [C, NSL], fp32, isOutput=False)
        kf = nc.declare_dram_parameter("kf", [C, N], fp32, isOutput=False)
        xf = nc.declare_dram_parameter("xf", [C, N], fp32, isOutput=False)
        xs = nc.declare_dram_parameter("xs", [C, NSL], fp32, isOutput=False)
        wqt = nc.declare_dram_parameter("wqt", [C, D], fp32, isOutput=False)
        wkt = nc.declare_dram_parameter("wkt", [C, D], fp32, isOutput=False)
        wvt = nc.declare_dram_parameter("wvt", [C, C], fp32, isOutput=False)
        bqr = nc.declare_dram_parameter("bqr", [1, D], fp32, isOutput=False)
        bkr = nc.declare_dram_parameter("bkr", [1, D], fp32, isOutput=False)
        bvr = nc.declare_dram_parameter("bvr", [1, E1], fp32, isOutput=False)
        outp = nc.declare_dram_parameter("outp", [C, NSL], fp32, isOutput=True)
        attn = nc.declare_dram_parameter("attn", [NSL, N], fp32, isOutput=True)
    else:
        # timing variant: same compute/DMA on internal DRAM, looped
        dum_i = nc.declare_dram_parameter("dum_i", [1, 4], fp32, isOutput=False)
        dum_o = nc.declare_dram_parameter("dum_o", [1, 4], fp32, isOutput=True)
        qs = nc.dram_tensor("qs", [C, NSL], fp32)
        kf = nc.dram_tensor("kf", [C, N], fp32)
        xf = nc.dram_tensor("xf", [C, N], fp32)
        xs = nc.dram_tensor("xs", [C, NSL], fp32)
        wqt = nc.dram_tensor("wqt", [C, D], fp32)
        wkt = nc.dram_tensor("wkt", [C, D], fp32)
        wvt = nc.dram_tensor("wvt", [C, C], fp32)
        bqr = nc.dram_tensor("bqr", [1, D], fp32)
        bkr = nc.dram_tensor("bkr", [1, D], fp32)
        bvr = nc.dram_tensor("bvr", [1, E1], fp32)
        outp = nc.dram_tensor("outp", [C, NSL], fp32)
        attn = nc.dram_tensor("attn", [NSL, N], fp32)

    with tile.TileContext(nc) as tc, ExitStack() as ctx:
        persist = ctx.enter_context(tc.tile_pool(name="persist", bufs=1))
        ident = persist.tile([128, 128], fp32)
        make_identity(nc, ident)
        ones_row = persist.tile([1, NCH], fp32)
        nc.vector.memset(ones_row, 1.0)

        # weights: loaded once (tiny)
        wvt_sb = persist.tile([128, 2, C], fp32)
        nc.sync.dma_start(out=wvt_sb, in_=wvt.rearrange("(t p) e -> p t e", p=128))
        wqt_sb = persist.tile([128, 2, D], fp32)
        nc.sync.dma_start(out=wqt_sb, in_=wqt.rearrange("(t p) d -> p t d", p=128))
        wkt_sb = persist.tile([128, 2, D], fp32)
        nc.sync.dma_start(out=wkt_sb, in_=wkt.rearrange("(t p) d -> p t d", p=128))
        bqr_sb = persist.tile([1, D], fp32)
        nc.sync.dma_start(out=bqr_sb, in_=bqr[:, :])
        bkr_sb = persist.tile([1, D], fp32)
        nc.sync.dma_start(out=bkr_sb, in_=bkr[:, :])
        bvr_sb = persist.tile([1, E1], fp32)
        nc.sync.dma_start(out=bvr_sb, in_=bvr[:, :])

        dpool = ctx.enter_context(tc.tile_pool(name="dpool", bufs=1))
        pspool = ctx.enter_context(
            tc.tile_pool(name="pspool", bufs=2, space="PSUM"))
        otpool = ctx.enter_context(
            tc.tile_pool(name="otpool", bufs=4, space="PSUM"))
        expool = ctx.enter_context(tc.tile_pool(name="expool", bufs=2))
        apool = ctx.enter_context(tc.tile_pool(name="apool", bufs=2))
        smpool = ctx.enter_context(tc.tile_pool(name="smpool", bufs=8))
        xspool = ctx.enter_context(tc.tile_pool(name="xspool", bufs=2))

        def body(it=0):
            # ---- phase A1: value projection (pvT_aug) ----
            xf_sb = dpool.tile([128, 2, N], fp32, tag="xf", name=f"xf{it}")
            nc.sync.dma_start(out=xf_sb, in_=xf.rearrange("(t p) n -> p t n", p=128))
            pvt_sb = dpool.tile([128, N // MT, E1], fp32, tag="pvt", name=f"pvt{it}")
            for mt in range(N // MT):
                ps = pspool.tile([128, E1], fp32, tag="e", name=f"pv{it}_{mt}")
                nc.tensor.matmul(
                    ps, ones_row[:, 0:128], bvr_sb, start=True, stop=False)
                for ct in range(2):
                    nc.tensor.matmul(
                        ps[:, 0:C],
                        xf_sb[:, ct, mt * MT:(mt + 1) * MT],
                        wvt_sb[:, ct, :],
                        start=False, stop=(ct == 1),
                    )
                nc.vector.tensor_copy(pvt_sb[:, mt, :], ps)

            # ---- phase A2: q/k projections ----
            qs_sb = dpool.tile([128, 2, NSL], fp32, tag="qs", name=f"qs{it}")
            nc.sync.dma_start(out=qs_sb, in_=qs.rearrange("(t p) n -> p t n", p=128))
            kf_sb = dpool.tile([128, 2, N], fp32, tag="kf", name=f"kf{it}")
            nc.sync.dma_start(out=kf_sb, in_=kf.rearrange("(t p) n -> p t n", p=128))
            pq_sb = dpool.tile([D, NSL], fp32, tag="pq", name=f"pq{it}")
            pk_sb = dpool.tile([D, N], fp32, tag="pk", name=f"pk{it}")

            for nch in range(NSL // NCH):
                ps = pspool.tile([D, NCH], fp32, tag="e", name=f"pjq{it}_{nch}")
                sl = slice(nch * NCH, (nch + 1) * NCH)
                nc.tensor.matmul(ps, bqr_sb, ones_row, start=True, stop=False)
                for ct in range(2):
                    nc.tensor.matmul(
                        ps, wqt_sb[:, ct, :], qs_sb[:, ct, sl],
                        start=False, stop=(ct == 1),
                    )
                nc.vector.tensor_copy(pq_sb[:, sl], ps)

            for mch in range(N // NCH):
                ps = pspool.tile([D, NCH], fp32, tag="e", name=f"pjk{it}_{mch}")
                sl = slice(mch * NCH, (mch + 1) * NCH)
                nc.tensor.matmul(ps, bkr_sb, ones_row, start=True, stop=False)
                for ct in range(2):
                    nc.tensor.matmul(
                        ps, wkt_sb[:, ct, :], kf_sb[:, ct, sl],
                        start=False, stop=(ct == 1),
                    )
                nc.vector.tensor_copy(pk_sb[:, sl], ps)

            out_sb = dpool.tile([128, 2, NSL], fp32, tag="out", name=f"out{it}")

            # ---- phase B ----
            for nch in range(NSL // NCH):
                nsl = slice(nch * NCH, (nch + 1) * NCH)
                ot = [otpool.tile([128, E1], fp32, tag="ot",
                                  name=f"ot{it}_{nch}_{i}") for i in range(4)]
                # energyT -> exp -> OT accumulate, 2 m-tiles per group
                for mg in range(N // (2 * MT)):
                    eps = pspool.tile([128, 2, NCH], fp32, tag="e",
                                      name=f"eT{it}_{nch}_{mg}")
                    for mi in range(2):
                        mt = mg * 2 + mi
                        nc.tensor.matmul(
                            eps[:, mi, :],
                            pk_sb[:, mt * MT:(mt + 1) * MT],
                            pq_sb[:, nsl],
                            start=True, stop=True,
                        )
                    ex = expool.tile([128, 2, NCH], fp32, tag="ex")
                    nc.scalar.activation(ex, eps, AF.Exp)
                    for mi in range(2):
                        mt = mg * 2 + mi
                        for ns in range(4):
                            nc.tensor.matmul(
                                ot[ns],
                                ex[:, mi, ns * NT:(ns + 1) * NT],
                                pvt_sb[:, mt, :],
                                start=(mt == 0), stop=(mt == N // MT - 1),
                            )

                for ns in range(4):
                    nt = nch * 4 + ns
                    s_col = ot[ns][:, C:E1]
                    rs = smpool.tile([128, 1], fp32, tag="sm")
                    nc.vector.reciprocal(rs, s_col)
                    rsg = smpool.tile([128, 1], fp32, tag="sm")
                    nc.vector.tensor_scalar_mul(rsg, rs, float(gamma))
                    lns = smpool.tile([128, 1], fp32, tag="sm")
                    nc.scalar.activation(lns, s_col, AF.Ln)
                    nls = smpool.tile([128, 1], fp32, tag="sm")
                    nc.vector.tensor_scalar_mul(nls, lns, -1.0)

                    otn = expool.tile([128, C], fp32, tag="otn")
                    nc.vector.tensor_scalar_mul(otn, ot[ns][:, 0:C], rsg)

                    # normalized attention rows: exp(E - lnS), 2KB-row halves
                    for half in range(2):
                        a_sb = apool.tile([128, N // 2], fp32, tag="a")
                        for mg in range(2):
                            eps = pspool.tile(
                                [128, 2, NCH], fp32, tag="e",
                                name=f"en{it}_{nt}_{half}_{mg}")
                            for mi in range(2):
                                mof = (half * 2 + mg) * 2 + mi
                                nc.tensor.matmul(
                                    eps[:, mi, :],
                                    pq_sb[:, nt * NT:(nt + 1) * NT],
                                    pk_sb[:, mof * NCH:(mof + 1) * NCH],
                                    start=True, stop=True,
                                )
                            nc.scalar.activation(
                                a_sb[:, mg * 2 * NCH:(mg + 1) * 2 * NCH]
                                .rearrange("p (a b) -> p a b", a=2),
                                eps, AF.Exp, bias=nls,
                            )
                        nc.sync.dma_start(
                            out=attn[nt * NT:(nt + 1) * NT,
                                     half * (N // 2):(half + 1) * (N // 2)],
                            in_=a_sb)

                    # out^T tile -> (c, n) + residual
                    xst = xspool.tile([128, 2, NT], fp32, tag="xs")
                    nc.sync.dma_start(
                        out=xst,
                        in_=xs.rearrange("(t p) n -> p t n", p=128)
                        [:, :, nt * NT:(nt + 1) * NT])
                    for ch in range(2):
                        tr = pspool.tile([128, 128], fp32, tag="e",
                                         name=f"tr{it}_{nt}_{ch}")
                        nc.tensor.transpose(
                            tr[:, 0:128], otn[:, ch * 128:(ch + 1) * 128], ident)
                        nc.vector.tensor_add(
                            out_sb[:, ch, nt * NT:(nt + 1) * NT],
                            tr[:, 0:128],
                            xst[:, ch, :],
                        )

            nc.sync.dma_start(
                out=outp.rearrange("(t p) n -> p t n", p=128), in_=out_sb)

        if loop_iters is None:
            body()
        else:
            with tc.For_i(0, loop_iters, 1):
                body()
            dum_sb = persist.tile([1, 4], fp32)
            nc.sync.dma_start(out=dum_sb, in_=dum_i[:, :])
            nc.sync.dma_start(out=dum_o[:, :], in_=dum_sb)

    _patch_serialization(nc)
    return nc


def _split_multiwaits(m):
    """This container's walrus accepts only one sem-wait per instruction;
    Tile's exit drain stacks several. Move extras onto EventSemaphore
    instructions inserted just before the offender."""
    for f in m["functions"]:
        for b in f.get("blocks", []):
            insts = b["instructions"]
            out = []
            for ins in insts:
                si = ins.get("sync_info")
                waits = (si or {}).get("on_wait") or []
                if len(waits) > 1:
                    for j, w in enumerate(waits[:-1]):
                        out.append({
                            "debug": ins.get("debug", 0),
                            "engine": ins["engine"],
                            "ins": [], "outs": [],
                            "name": f"{ins['name']}_sw{j}",
                            "opcode": "EventSemaphore",
                            "sync_info": {"on_update": [], "on_wait": [w]},
                        })
                    si["on_wait"] = [waits[-1]]
                out.append(ins)
            b["instructions"] = out


def _patch_serialization(nc):
    orig = nc.to_json_bytes

    def patched():
        m = json.loads(orig())
        _split_multiwaits(m)
        return json.dumps(m).encode()

    nc.to_json_bytes = patched


def _shard_inputs(x, k, q, Wq, bq, Wk, bk, Wv, bv):
    """Host-side slicing into the 8 per-core input maps."""
    cc = np.ascontiguousarray
    maps = []
    for core in range(8):
        b, h = core // 2, core % 2
        nsl = slice(h * NSL, (h + 1) * NSL)
        qf = q[b].reshape(C, N)
        kfb = k[b].reshape(C, N)
        xfb = x[b].reshape(C, N)
        bvr = np.empty((1, E1), np.float32)
        bvr[0, :C] = bv
        bvr[0, C] = 1.0
        maps.append({
            "qs": cc(qf[:, nsl]),
            "kf": cc(kfb),
            "xf": cc(xfb),
            "xs": cc(xfb[:, nsl]),
            "wqt": cc(Wq.T),
            "wkt": cc(Wk.T),
            "wvt": cc(Wv.T),
            "bqr": cc(bq.reshape(1, D)),
            "bkr": cc(bk.reshape(1, D)),
            "bvr": bvr,
        })
    return maps


def kernel(x, k, q, Wq, bq, Wk, bk, Wv, bv, gamma):
    from concourse.bass_utils import run_bass_kernel_spmd

    x, k, q = np.asarray(x, np.float32), np.asarray(k, np.float32), np.asarray(q, np.float32)
    Wq, bq = np.asarray(Wq, np.float32), np.asarray(bq, np.float32)
    Wk, bk = np.asarray(Wk, np.float32), np.asarray(bk, np.float32)
    Wv, bv = np.asarray(Wv, np.float32), np.asarray(bv, np.float32)
    g = float(np.asarray(gamma).reshape(-1)[0])

    key = ("main", g)
    if key not in _CACHE:
        _CACHE[key] = _build(g)
    nc = _CACHE[key]

    maps = _shard_inputs(x, k, q, Wq, bq, Wk, bk, Wv, bv)
    res = run_bass_kernel_spmd(nc, maps, core_ids=list(range(8)))

    out = np.empty((B, C, N), np.float32)
    att = np.empty((B, N, N), np.float32)
    for core in range(8):
        b, h = core // 2, core % 2
        nsl = slice(h * NSL, (h + 1) * NSL)
        out[b][:, nsl] = res.results[core]["outp"]
        att[b][nsl, :] = res.results[core]["attn"]
    return out.reshape(B, C, 64, 64), att


# revision 12
# speedup vs baseline: 2.1806x; 2.1806x over previous
"""Trainium2 Bass kernel for nn_Cond_Attn (B=4, C=256, N=4096, d=8).

Sharding: 8 cores; core i handles batch i//2, attention-row half i%2
(2048 of 4096 rows). Each core computes its full attention row-slice
(softmax over all 4096 keys is local) plus the matching out columns.

Per-core pipeline (all fp32):
  - pq (8,2048), pk (8,4096): tiny projections via PE (weights passed
    pre-transposed from host; biases folded in as K=1 matmuls).
  - pvT_aug (4096, 257): value projection, transposed layout, with a
    ones column at index 256 so the softmax denominator S falls out of
    the OT matmul for free.
  - per 512-wide n-chunk: energyT tiles (m,n) -> ACT exp -> OT matmuls
    (lhsT = expET 128x128 subtiles, rhs = pvT_aug) accumulate
    OT (n,257) in PSUM: cols 0:256 = unnormalized out^T, col 256 = S.
  - per 128-row n-tile: recompute energy in (n,m) layout and apply
    exp(E - lnS) on ACT (per-partition bias) -> normalized attention,
    DMA'd out in 1MB blocks. OT is scaled by gamma/S, PE-transposed to
    (c,n), residual x added, DMA'd out.
"""
import sys

if "/opt/trn_rl_repo" not in sys.path:
    sys.path.insert(0, "/opt/trn_rl_repo")

import json
from contextlib import ExitStack

import numpy as np

B, C, N, D = 4, 256, 4096, 8
NSL = N // 2          # n rows per core
NCH = 512             # n-chunk width
NT = 128              # n-tile (partition block)
MT = 128              # m-tile
E1 = 258              # pvT width: 256 pv + ones col + pad (f32r needs even N)

_CACHE = {}


def _build(gamma: float, loop_iters: int | None = None, raw_loads: bool = False):
    import concourse.bass as bass
    import concourse.mybir as mybir
    import concourse.tile as tile
    from concourse.masks import make_identity

    fp32 = mybir.dt.float32
    f32r = mybir.dt.float32r
    AF = mybir.ActivationFunctionType

    nc = bass.Bass()
    if loop_iters is None:
        qs = nc.declare_dram_parameter("qs", [C, NSL], fp32, isOutput=False)
        kf = nc.declare_dram_parameter("kf", [C, N], fp32, isOutput=False)
        xf = nc.declare_dram_parameter("xf", [C, N], fp32, isOutput=False)
        xs = nc.declare_dram_parameter("xs", [C, NSL], fp32, isOutput=False)
        wqt = nc.declare_dram_parameter("wqt", [C, D], fp32, isOutput=False)
        wkt = nc.declare_dram_parameter("wkt", [C, D], fp32, isOutput=False)
        wvt = nc.declare_dram_parameter("wvt", [C, C], fp32, isOutput=False)
        bqr = nc.declare_dram_parameter("bqr", [1, D], fp32, isOutput=False)
        bkr = nc.declare_dram_parameter("bkr", [1, D], fp32, isOutput=False)
        bvr = nc.declare_dram_parameter("bvr", [1, E1], fp32, isOutput=False)
        outp = nc.declare_dram_parameter("outp", [C, NSL], fp32, isOutput=True)
        attn = nc.declare_dram_parameter("attn", [NSL, N], fp32, isOutput=True)
    else:
        # timing variant: same compute/DMA on internal DRAM, looped
        dum_i = nc.declare_dram_parameter("dum_i", [1, 4], fp32, isOutput=False)
        dum_o = nc.declare_dram_parameter("dum_o", [1, 4], fp32, isOutput=True)
        qs = nc.dram_tensor("qs", [C, NSL], fp32)
        kf = nc.dram_tensor("kf", [C, N], fp32)
        xf = nc.dram_tensor("xf", [C, N], fp32)
        xs = nc.dram_tensor("xs", [C, NSL], fp32)
        wqt = nc.dram_tensor("wqt", [C, D], fp32)
        wkt = nc.dram_tensor("wkt", [C, D], fp32)
        wvt = nc.dram_tensor("wvt", [C, C], fp32)
        bqr = nc.dram_tensor("bqr", [1, D], fp32)
        bkr = nc.dram_tensor("bkr", [1, D], fp32)
        bvr = nc.dram_tensor("bvr", [1, E1], fp32)
        outp = nc.dram_tensor("outp", [C, NSL], fp32)
        attn = nc.dram_tensor("attn", [NSL, N], fp32)

    def load(out_tile, in_ap):
        if raw_loads:
            nc.sync.dma_start(out=out_tile.bitcast(fp32), in_=in_ap)
        else:
            nc.gpsimd.dma_start(out=out_tile, in_=in_ap)

    with tile.TileContext(nc) as tc, ExitStack() as ctx:
        persist = ctx.enter_context(tc.tile_pool(name="persist", bufs=1))
        ident = persist.tile([128, 128], fp32)
        make_identity(nc, ident)
        ones_fp = persist.tile([1, NCH], fp32)
        nc.vector.memset(ones_fp, 1.0)
        ones_row = persist.tile([1, NCH], f32r)
        nc.vector.tensor_copy(ones_row, ones_fp)

        # weights: loaded once (tiny)
        wvt_sb = persist.tile([128, 2, C], f32r)
        load(wvt_sb, wvt.rearrange("(t p) e -> p t e", p=128))
        wqt_sb = persist.tile([128, 2, D], f32r)
        load(wqt_sb, wqt.rearrange("(t p) d -> p t d", p=128))
        wkt_sb = persist.tile([128, 2, D], f32r)
        load(wkt_sb, wkt.rearrange("(t p) d -> p t d", p=128))
        bqr_sb = persist.tile([1, D], f32r)
        load(bqr_sb, bqr[:, :])
        bkr_sb = persist.tile([1, D], f32r)
        load(bkr_sb, bkr[:, :])
        bvr_sb = persist.tile([1, E1], f32r)
        load(bvr_sb, bvr[:, :])

        dpool = ctx.enter_context(tc.tile_pool(name="dpool", bufs=1))
        pspool = ctx.enter_context(
            tc.tile_pool(name="pspool", bufs=2, space="PSUM"))
        otpool = ctx.enter_context(
            tc.tile_pool(name="otpool", bufs=4, space="PSUM"))
        expool = ctx.enter_context(tc.tile_pool(name="expool", bufs=2))
        apool = ctx.enter_context(tc.tile_pool(name="apool", bufs=2))
        smpool = ctx.enter_context(tc.tile_pool(name="smpool", bufs=8))
        xspool = ctx.enter_context(tc.tile_pool(name="xspool", bufs=2))

        def body(it=0):
            # ---- phase A1: value projection (pvT_aug) ----
            xf_sb = dpool.tile([128, 2, N], f32r, tag="xf", name=f"xf{it}")
            load(xf_sb, xf.rearrange("(t p) n -> p t n", p=128))
            pvt_sb = dpool.tile([128, N // MT, E1], f32r, tag="pvt", name=f"pvt{it}")
            for mt in range(N // MT):
                ps = pspool.tile([128, E1], fp32, tag="e", name=f"pv{it}_{mt}")
                nc.tensor.matmul(
                    ps, ones_row[:, 0:128], bvr_sb, start=True, stop=False)
                for ct in range(2):
                    nc.tensor.matmul(
                        ps[:, 0:C],
                        xf_sb[:, ct, mt * MT:(mt + 1) * MT],
                        wvt_sb[:, ct, :],
                        start=False, stop=(ct == 1),
                    )
                nc.vector.tensor_copy(pvt_sb[:, mt, :], ps)

            # ---- phase A2: q/k projections ----
            qs_sb = dpool.tile([128, 2, NSL], f32r, tag="qs", name=f"qs{it}")
            load(qs_sb, qs.rearrange("(t p) n -> p t n", p=128))
            kf_sb = dpool.tile([128, 2, N], f32r, tag="kf", name=f"kf{it}")
            load(kf_sb, kf.rearrange("(t p) n -> p t n", p=128))
            pq_sb = dpool.tile([D, NSL], f32r, tag="pq", name=f"pq{it}")
            pk_sb = dpool.tile([D, N], f32r, tag="pk", name=f"pk{it}")

            for nch in range(NSL // NCH):
                ps = pspool.tile([D, NCH], fp32, tag="e", name=f"pjq{it}_{nch}")
                sl = slice(nch * NCH, (nch + 1) * NCH)
                nc.tensor.matmul(ps, bqr_sb, ones_row, start=True, stop=False)
                for ct in range(2):
                    nc.tensor.matmul(
                        ps, wqt_sb[:, ct, :], qs_sb[:, ct, sl],
                        start=False, stop=(ct == 1),
                    )
                nc.vector.tensor_copy(pq_sb[:, sl], ps)

            for mch in range(N // NCH):
                ps = pspool.tile([D, NCH], fp32, tag="e", name=f"pjk{it}_{mch}")
                sl = slice(mch * NCH, (mch + 1) * NCH)
                nc.tensor.matmul(ps, bkr_sb, ones_row, start=True, stop=False)
                for ct in range(2):
                    nc.tensor.matmul(
                        ps, wkt_sb[:, ct, :], kf_sb[:, ct, sl],
                        start=False, stop=(ct == 1),
                    )
                nc.vector.tensor_copy(pk_sb[:, sl], ps)

            out_sb = dpool.tile([128, 2, NSL], fp32, tag="out", name=f"out{it}")

            # ---- phase B ----
            for nch in range(NSL // NCH):
                nsl = slice(nch * NCH, (nch + 1) * NCH)
                ot = [otpool.tile([128, E1], fp32, tag="ot",
                                  name=f"ot{it}_{nch}_{i}") for i in range(4)]
                # energyT -> exp -> OT accumulate, 2 m-tiles per group
                for mg in range(N // (2 * MT)):
                    eps = pspool.tile([128, 2, NCH], fp32, tag="e",
                                      name=f"eT{it}_{nch}_{mg}")
                    for mi in range(2):
                        mt = mg * 2 + mi
                        nc.tensor.matmul(
                            eps[:, mi, :],
                            pk_sb[:, mt * MT:(mt + 1) * MT],
                            pq_sb[:, nsl],
                            start=True, stop=True,
                        )
                    ex = expool.tile([128, 2, NCH], f32r, tag="ex")
                    nc.scalar.activation(ex, eps, AF.Exp)
                    for mi in range(2):
                        mt = mg * 2 + mi
                        for ns in range(4):
                            nc.tensor.matmul(
                                ot[ns],
                                ex[:, mi, ns * NT:(ns + 1) * NT],
                                pvt_sb[:, mt, :],
                                start=(mt == 0), stop=(mt == N // MT - 1),
                            )

                for ns in range(4):
                    nt = nch * 4 + ns
                    s_col = ot[ns][:, C:C + 1]
                    rs = smpool.tile([128, 1], fp32, tag="sm")
                    nc.vector.reciprocal(rs, s_col)
                    rsg = smpool.tile([128, 1], fp32, tag="sm")
                    nc.vector.tensor_scalar_mul(rsg, rs, float(gamma))
                    lns = smpool.tile([128, 1], fp32, tag="sm")
                    nc.scalar.activation(lns, s_col, AF.Ln)
                    nls = smpool.tile([128, 1], fp32, tag="sm")
                    nc.vector.tensor_scalar_mul(nls, lns, -1.0)

                    otn = expool.tile([128, C], fp32, tag="otn")
                    nc.vector.tensor_scalar_mul(otn, ot[ns][:, 0:C], rsg)

                    # normalized attention rows: exp(E - lnS), 2KB-row halves
                    for half in range(2):
                        a_sb = apool.tile([128, N // 2], fp32, tag="a")
                        for mg in range(2):
                            eps = pspool.tile(
                                [128, 2, NCH], fp32, tag="e",
                                name=f"en{it}_{nt}_{half}_{mg}")
                            for mi in range(2):
                                mof = (half * 2 + mg) * 2 + mi
                                nc.tensor.matmul(
                                    eps[:, mi, :],
                                    pq_sb[:, nt * NT:(nt + 1) * NT],
                                    pk_sb[:, mof * NCH:(mof + 1) * NCH],
                                    start=True, stop=True,
                                )
                            nc.scalar.activation(
                                a_sb[:, mg * 2 * NCH:(mg + 1) * 2 * NCH]
                                .rearrange("p (a b) -> p a b", a=2),
                                eps, AF.Exp, bias=nls,
                            )
                        nc.sync.dma_start(
                            out=attn[nt * NT:(nt + 1) * NT,
                                     half * (N // 2):(half + 1) * (N // 2)],
                            in_=a_sb)

                    # out^T tile -> (c, n) + residual
                    xst = xspool.tile([128, 2, NT], fp32, tag="xs")
                    nc.sync.dma_start(
                        out=xst,
                        in_=xs.rearrange("(t p) n -> p t n", p=128)
                        [:, :, nt * NT:(nt + 1) * NT])
                    for ch in range(2):
                        tr = pspool.tile([128, 128], fp32, tag="e",
                                         name=f"tr{it}_{nt}_{ch}")
                        nc.tensor.transpose(
                            tr[:, 0:128], otn[:, ch * 128:(ch + 1) * 128], ident)
                        nc.vector.tensor_add(
                            out_sb[:, ch, nt * NT:(nt + 1) * NT],
                            tr[:, 0:128],
                            xst[:, ch, :],
                        )

            nc.sync.dma_start(
                out=outp.rearrange("(t p) n -> p t n", p=128), in_=out_sb)

        if loop_iters is None:
            body()
        else:
            with tc.For_i(0, loop_iters, 1):
                body()
            dum_sb = persist.tile([1, 4], fp32)
            nc.sync.dma_start(out=dum_sb, in_=dum_i[:, :])
            nc.sync.dma_start(out=dum_o[:, :], in_=dum_sb)

    _patch_serialization(nc)
    return nc


def _split_multiwaits(m):
    """This container's walrus accepts only one sem-wait per instruction;
    Tile's exit drain stacks several. Move extras onto EventSemaphore
    instructions inserted just before the offender."""
    for f in m["functions"]:
        for b in f.get("blocks", []):
            insts = b["instructions"]
            out = []
            for ins in insts:
                si = ins.get("sync_info")
                waits = (si or {}).get("on_wait") or []
                if len(waits) > 1:
                    for j, w in enumerate(waits[:-1]):
                        out.append({
                            "debug": ins.get("debug", 0),
                            "engine": ins["engine"],
                            "ins": [], "outs": [],
                            "name": f"{ins['name']}_sw{j}",
                            "opcode": "EventSemaphore",
                            "sync_info": {"on_update": [], "on_wait": [w]},
                        })
                    si["on_wait"] = [waits[-1]]
                out.append(ins)
            b["instructions"] = out


def _patch_serialization(nc):
    orig = nc.to_json_bytes

    def patched():
        m = json.loads(orig())
        _split_multiwaits(m)
        return json.dumps(m).encode()

    nc.to_json_bytes = patched


def _shard_inputs(x, k, q, Wq, bq, Wk, bk, Wv, bv):
    """Host-side slicing into the 8 per-core input maps."""
    cc = np.ascontiguousarray
    maps = []
    for core in range(8):
        b, h = core // 2, core % 2
        nsl = slice(h * NSL, (h + 1) * NSL)
        qf = q[b].reshape(C, N)
        kfb = k[b].reshape(C, N)
        xfb = x[b].reshape(C, N)
        bvr = np.zeros((1, E1), np.float32)
        bvr[0, :C] = bv
        bvr[0, C] = 1.0
        maps.append({
            "qs": cc(qf[:, nsl]),
            "kf": cc(kfb),
            "xf": cc(xfb),
            "xs": cc(xfb[:, nsl]),
            "wqt": cc(Wq.T),
            "wkt": cc(Wk.T),
            "wvt": cc(Wv.T),
            "bqr": cc(bq.reshape(1, D)),
            "bkr": cc(bk.reshape(1, D)),
            "bvr": bvr,
        })
    return maps


def kernel(x, k, q, Wq, bq, Wk, bk, Wv, bv, gamma):
    from concourse.bass_utils import run_bass_kernel_spmd

    x, k, q = np.asarray(x, np.float32), np.asarray(k, np.float32), np.asarray(q, np.float32)
    Wq, bq = np.asarray(Wq, np.float32), np.asarray(bq, np.float32)
    Wk, bk = np.asarray(Wk, np.float32), np.asarray(bk, np.float32)
    Wv, bv = np.asarray(Wv, np.float32), np.asarray(bv, np.float32)
    g = float(np.asarray(gamma).reshape(-1)[0])

    key = ("main", g)
    if key not in _CACHE:
        _CACHE[key] = _build(g)
    nc = _CACHE[key]

    maps = _shard_inputs(x, k, q, Wq, bq, Wk, bk, Wv, bv)
    res = run_bass_kernel_spmd(nc, maps, core_ids=list(range(8)))

    out = np.empty((B, C, N), np.float32)
    att = np.empty((B, N, N), np.float32)
    for core in range(8):
        b, h = core // 2, core % 2
        nsl = slice(h * NSL, (h + 1) * NSL)
        out[b][:, nsl] = res.results[core]["outp"]
        att[b][nsl, :] = res.results[core]["attn"]
    return out.reshape(B, C, 64, 64), att
